# revision 25
# baseline (speedup 1.0000x reference)
"""nn_HHLTraj Bass/Tile kernel for 8 trn2 NeuronCores (single NEFF).

Graph stage (hypergraph conv, factorized — G is never materialized):
    G @ Y = A_left @ (A_right @ Y),  A_left = DV2*Hbat^T [N,E],
    A_right = invDE*Hbat*DV2 [E,N].
  n-contraction sharded: core c holds A_right^T row-slice R_c [NLOC,E] and
  A_left^T col-slice L_c [E,NLOC]; z-partials are AllReduced ([H,E] 1MB),
  x-rows are computed locally, final x2^T is AllGathered. hgc biases enter
  as rank-1 matmuls (b ⊗ rowsum(A_right)/8), so Y1/Y2 GEMMs are bias-free.

GRU stage (3 layers, T=128 steps, B-sharded 64 seqs/core):
  layer wavefront with skew D supersteps; per-layer independent dependency
  chains (no cross-layer op merging). Gate preactivations accumulate in PSUM
  on top of the chunked gx GEMM output (no DVE add for r/z). z-gate is
  host-negated so z' = 1-z = sigmoid(-a_z) and h' = h + z'*(n - h).
  Per-step biases enter via ACT per-partition bias (sigmoid) and
  scalar_tensor_tensor per-partition scalars (n-gate). Elementwise is split
  across DVE / ACT / GPSIMD; gathers (emb, final step) use GPSIMD ap_gather
  against SBUF-resident x2^T / out2.

Numerics: all matmuls fp32 (bf16/tf32 operand rounding is amplified ~1000x
by this network — measured rel_err 0.5-2.0 vs the 2e-2 gate). fp32 h state
and out2 storage.
"""
import sys

import numpy as np

if "/opt/trn_rl_repo" not in sys.path:
    sys.path.insert(0, "/opt/trn_rl_repo")

N, E, B, T, H, L = 8192, 2048, 512, 128, 128, 3
NC = 8
NLOC = N // NC    # 1024
BLOC = B // NC    # 64
D = 5             # wavefront layer skew (supersteps)
CH = 4            # gx chunk length (steps)
SS = T + (L - 1) * D

_cache = {}


def _wrap_idx16(idx):
    """ap_gather index layout: idxs[p, s] = idx[s*16 + p%16] (all 8 Q7 groups
    get the same indices)."""
    idx = np.asarray(idx, np.int64)
    n = idx.shape[0]
    assert n % 16 == 0
    out = np.zeros((128, n // 16), np.int16)
    s = np.arange(n // 16)
    for p in range(128):
        out[p, :] = idx[s * 16 + (p % 16)]
    return out


def build(stub_collectives=False):
    import concourse.bacc as bacc
    import concourse.mybir as mybir
    import concourse.tile as tile
    from concourse.masks import make_identity

    f32 = mybir.dt.float32
    i16 = mybir.dt.int16

    nc = bacc.Bacc(num_devices=NC)

    # ---------------- DRAM parameters ----------------
    R_d = nc.declare_dram_parameter("R", [NLOC, E], f32, isOutput=False)
    L_d = nc.declare_dram_parameter("Lm", [E, NLOC], f32, isOutput=False)
    poiT_d = nc.declare_dram_parameter("poiT", [H, NLOC], f32, isOutput=False)
    W1_d = nc.declare_dram_parameter("W1", [H, H], f32, isOutput=False)
    W2_d = nc.declare_dram_parameter("W2", [H, H], f32, isOutput=False)
    rsb_d = nc.declare_dram_parameter("rsb", [1, E], f32, isOutput=False)
    b1_d = nc.declare_dram_parameter("b1", [1, H], f32, isOutput=False)
    b2_d = nc.declare_dram_parameter("b2", [1, H], f32, isOutput=False)
    wih_d = nc.declare_dram_parameter("WIH", [H, L * 3 * H], f32, isOutput=False)
    whh_d = nc.declare_dram_parameter("WHH", [H, L * 3 * H], f32, isOutput=False)
    brz_d = nc.declare_dram_parameter("BRZ2", [H, 2 * L], f32, isOutput=False)
    bxn_d = nc.declare_dram_parameter("BXN", [H, L], f32, isOutput=False)
    bhn_d = nc.declare_dram_parameter("BHN2", [H, L], f32, isOutput=False)
    eidx_d = nc.declare_dram_parameter("EIDX", [128, T * BLOC // 16], i16, isOutput=False)
    fidx_d = nc.declare_dram_parameter("FIDX", [128, BLOC // 16], i16, isOutput=False)
    out_d = nc.declare_dram_parameter("out", [BLOC, H], f32, isOutput=True)

    # internal DRAM for collectives
    z_in = nc.dram_tensor("z_in", [H, E], f32)
    z_out = nc.dram_tensor("z_out", [H, E], f32, addr_space="Shared")
    z2_in = nc.dram_tensor("z2_in", [H, E], f32)
    z2_out = nc.dram_tensor("z2_out", [H, E], f32, addr_space="Shared")
    ag_in = nc.dram_tensor("ag_in", [NLOC, H], f32)
    ag_out = nc.dram_tensor("ag_out", [N, H], f32, addr_space="Shared")

    rg = [list(range(NC))]
    AL = mybir.AluOpType
    AF = mybir.ActivationFunctionType

    with tile.TileContext(nc) as tc:
        # ================= graph stage =================
        with tc.tile_pool(name="gwt", bufs=1) as gwt, \
             tc.tile_pool(name="gbig", bufs=1) as gbig, \
             tc.tile_pool(name="gsb", bufs=1) as gsb, \
             tc.tile_pool(name="gps", bufs=2, space="PSUM") as gps, \
             tc.tile_pool(name="gpz", bufs=1, space="PSUM") as gpz:

            poiT_t = gwt.tile([H, NLOC], f32, tag="poiT")
            W1_t = gwt.tile([H, H], f32, tag="W1")
            W2_t = gwt.tile([H, H], f32, tag="W2")
            rsb_t = gwt.tile([1, E], f32, tag="rsb")
            b1_t = gwt.tile([1, H], f32, tag="b1")
            b2_t = gwt.tile([1, H], f32, tag="b2")
            ident = gwt.tile([H, H], f32, tag="ident")
            nc.sync.dma_start(out=poiT_t, in_=poiT_d[:, :])
            nc.sync.dma_start(out=W1_t, in_=W1_d[:, :])
            nc.sync.dma_start(out=W2_t, in_=W2_d[:, :])
            nc.sync.dma_start(out=rsb_t, in_=rsb_d[:, :])
            nc.sync.dma_start(out=b1_t, in_=b1_d[:, :])
            nc.sync.dma_start(out=b2_t, in_=b2_d[:, :])
            make_identity(nc, ident)

            Rt = gbig.tile([128, 8, E], f32, tag="Rt")
            Lt = gbig.tile([128, 16, NLOC], f32, tag="Lt")
            R_view = R_d.ap().rearrange("(k p) e -> p k e", p=128)
            L_view = L_d.ap().rearrange("(k p) n -> p k n", p=128)
            for k in range(8):
                nc.sync.dma_start(out=Rt[:, k, :], in_=R_view[:, k, :])
            for k in range(16):
                nc.sync.dma_start(out=Lt[:, k, :], in_=L_view[:, k, :])

            # Y1 = poi @ W1 (rows of this core, [128, 8, H], partition = n%128)
            Y1_t = gbig.tile([128, 8, H], f32, tag="Y1")
            for k in range(8):
                py = gps.tile([128, H], f32, tag="pp")
                nc.tensor.matmul(py, poiT_t[:, k * 128:(k + 1) * 128], W1_t,
                                 start=True, stop=True)
                nc.vector.tensor_copy(Y1_t[:, k, :], py)

            def z_gemm(Ysrc, btile, zin_dram, zout_dram, tagp):
                # z^T partial [H, E] = sum_k Ysrc_k^T @ R_k  (+ rank-1 b⊗rs)
                pz = gpz.tile([128, E], f32, tag="pz")
                for k in range(8):
                    for j in range(4):
                        nc.tensor.matmul(pz[:, j * 512:(j + 1) * 512],
                                         Ysrc[:, k, :], Rt[:, k, j * 512:(j + 1) * 512],
                                         start=(k == 0), stop=False,
                                         skip_group_check=True)
                for j in range(4):
                    nc.tensor.matmul(pz[:, j * 512:(j + 1) * 512], btile,
                                     rsb_t[:, j * 512:(j + 1) * 512],
                                     start=False, stop=True, skip_group_check=True)
                zT_sb = gsb.tile([H, E], f32, tag="zT")
                for j in range(2):
                    nc.vector.tensor_copy(zT_sb[:, j * 1024:(j + 1) * 1024],
                                          pz[:, j * 1024:(j + 1) * 1024])
                nc.sync.dma_start(out=zin_dram[:, :], in_=zT_sb)
                if stub_collectives:
                    nc.sync.dma_start(out=zout_dram[:, :], in_=zin_dram[:, :])
                else:
                    nc.gpsimd.collective_compute(
                        "AllReduce", AL.add, replica_groups=rg,
                        ins=[zin_dram.ap()], outs=[zout_dram.ap()])
                zTf = gsb.tile([H, E], f32, tag="zT")
                nc.sync.dma_start(out=zTf, in_=zout_dram[:, :])
                # transpose -> z tiles [128, 16, H] (partition = e%128)
                zt = gsb.tile([128, 16, H], f32, tag="zt")
                for j in range(16):
                    pt = gps.tile([128, H], f32, tag="pp")
                    nc.tensor.transpose(pt, zTf[:, j * 128:(j + 1) * 128], ident)
                    if j % 2 == 0:
                        nc.vector.tensor_copy(zt[:, j, :], pt)
                    else:
                        nc.scalar.copy(zt[:, j, :], pt)
                return zt

            def x_gemm(zt):
                # x^T [H, NLOC] = sum_k z_k^T(lhsT) @ L_k
                px = gpz.tile([128, NLOC], f32, tag="px")
                for k in range(16):
                    for j in range(2):
                        nc.tensor.matmul(px[:, j * 512:(j + 1) * 512],
                                         zt[:, k, :], Lt[:, k, j * 512:(j + 1) * 512],
                                         start=(k == 0), stop=(k == 15),
                                         skip_group_check=True)
                return px

            z1t = z_gemm(Y1_t, b1_t, z_in, z_out, "z1")
            px1 = x_gemm(z1t)
            # x1^T = relu(px1) + poi^T
            x1T = gsb.tile([H, NLOC], f32, tag="x1T")
            nc.vector.scalar_tensor_tensor(out=x1T, in0=px1, scalar=0.0,
                                           in1=poiT_t, op0=AL.max, op1=AL.add)
            # Y2 = x1 @ W2  [128, 8, H]
            Y2_t = gbig.tile([128, 8, H], f32, tag="Y2")
            for k in range(8):
                py = gps.tile([128, H], f32, tag="pp")
                nc.tensor.matmul(py, x1T[:, k * 128:(k + 1) * 128], W2_t,
                                 start=True, stop=True)
                nc.vector.tensor_copy(Y2_t[:, k, :], py)
            z2t = z_gemm(Y2_t, b2_t, z2_in, z2_out, "z2")
            px2 = x_gemm(z2t)
            x2T_c = gsb.tile([H, NLOC], f32, tag="x1T")
            for j in range(2):
                nc.vector.tensor_copy(x2T_c[:, j * 512:(j + 1) * 512],
                                      px2[:, j * 512:(j + 1) * 512])
            # transpose to row-major x2 [NLOC, H] so emb rows can be
            # dma-gathered from DRAM after the AllGather
            x2row = gsb.tile([128, 8, H], f32, tag="zt")
            for j in range(8):
                pt = gps.tile([128, H], f32, tag="pp")
                nc.tensor.transpose(pt, x2T_c[:, j * 128:(j + 1) * 128], ident)
                if j % 2 == 0:
                    nc.vector.tensor_copy(x2row[:, j, :], pt)
                else:
                    nc.scalar.copy(x2row[:, j, :], pt)
            nc.sync.dma_start(
                out=ag_in.ap().rearrange("(j p) h -> p j h", p=128), in_=x2row)
            if stub_collectives:
                for c in range(NC):
                    nc.sync.dma_start(out=ag_out[c * NLOC:(c + 1) * NLOC, :],
                                      in_=ag_in[:, :])
            else:
                nc.gpsimd.collective_compute(
                    "AllGather", AL.bypass, replica_groups=rg,
                    ins=[ag_in.ap()], outs=[ag_out.ap()])

        # ================= GRU stage =================
        with tc.tile_pool(name="rwt", bufs=1) as rwt, \
             tc.tile_pool(name="rbig", bufs=1) as rbig, \
             tc.tile_pool(name="rsb", bufs=4) as rsb, \
             tc.tile_pool(name="cps", bufs=1, space="PSUM") as cps:

            wih_t = rwt.tile([H, L * 3 * H], f32, tag="wih")
            whh_t = rwt.tile([H, L * 3 * H], f32, tag="whh")
            brz_t = rwt.tile([H, 2 * L], f32, tag="brz")
            bxn_t = rwt.tile([H, L], f32, tag="bxn")
            bhn_t = rwt.tile([H, L], f32, tag="bhn")
            eidx_t = rwt.tile([128, T * BLOC // 16], i16, tag="eidx")
            fidx_t = rwt.tile([128, BLOC // 16], i16, tag="fidx")
            nc.sync.dma_start(out=wih_t, in_=wih_d[:, :])
            nc.sync.dma_start(out=whh_t, in_=whh_d[:, :])
            nc.sync.dma_start(out=brz_t, in_=brz_d[:, :])
            nc.sync.dma_start(out=bxn_t, in_=bxn_d[:, :])
            nc.sync.dma_start(out=bhn_t, in_=bhn_d[:, :])
            nc.sync.dma_start(out=eidx_t, in_=eidx_d[:, :])
            nc.sync.dma_start(out=fidx_t, in_=fidx_d[:, :])

            embT = rbig.tile([128, T * BLOC], f32, tag="embT")
            inp1 = rbig.tile([128, T * BLOC], f32, tag="inp1")
            inp2 = rbig.tile([128, T * BLOC], f32, tag="inp2")
            out2 = rbig.tile([128, T * BLOC], f32, tag="out2")
            inps = [embT, inp1, inp2]

            ident2 = rwt.tile([H, H], f32, tag="ident2")
            make_identity(nc, ident2)

            hA = rwt.tile([128, 2 * BLOC], f32, tag="hA")   # layers 0,1 ping
            hB = rwt.tile([128, 2 * BLOC], f32, tag="hB")   # layers 0,1 pong
            zro = rwt.tile([128, BLOC], f32, tag="zro")
            nc.vector.memset(hA, 0.0)
            nc.vector.memset(hB, 0.0)
            nc.vector.memset(zro, 0.0)

            # persistent PSUM (7 banks):
            #  - per-layer gx-rz chunk gcp [1 bank each]
            #  - per-layer step tile sxp = [a_rz(128) | xn-chunk(256)] [1 bank]
            #  - hntp = [hn0|hn1|hn2(192) | transpose scratch(128)] [1 bank]
            # gx is injected into a_rz by an identity matmul, so the sigmoid
            # reads the complete preactivation straight from PSUM (no DVE add)
            gcp = [cps.tile([128, 2, CH * BLOC], f32, tag=f"gc{l}", name=f"gcp{l}")
                   for l in range(L)]
            sxp = [cps.tile([128, 2 * BLOC + CH * BLOC], f32, tag=f"sx{l}",
                            name=f"sxp{l}") for l in range(L)]
            hntp = cps.tile([128, L * BLOC + H], f32, tag="hntp")
            tpp = hntp[:, L * BLOC:L * BLOC + H]
            hnsl = [hntp[:, l * BLOC:(l + 1) * BLOC] for l in range(L)]
            xnsl = [sxp[l][:, 2 * BLOC:] for l in range(L)]

            def emit_emb_piece(g):
                # gather 1024 emb rows from row-major x2 in DRAM, then
                # transpose 128-row blocks into embT columns
                er = rbig.tile([128, 8, H], f32, bufs=2, tag="embr",
                               name=f"embr{g}")
                nc.gpsimd.dma_gather(er, ag_out.ap(),
                                     eidx_t[:, g * 64:(g + 1) * 64],
                                     num_idxs=1024, num_idxs_reg=1024,
                                     elem_size=H)
                for j in range(8):
                    nc.tensor.transpose(tpp, er[:, j, :], ident2)
                    dst = embT[:, g * 1024 + j * 128:g * 1024 + (j + 1) * 128]
                    if j % 2 == 0:
                        nc.vector.tensor_copy(dst, tpp)
                    else:
                        nc.scalar.copy(dst, tpp)

            emit_emb_piece(0)

            gxs = [None] * L
            wihs = [[wih_t[:, (l * 3 + g) * H:(l * 3 + g + 1) * H] for g in range(3)]
                    for l in range(L)]
            whhs = [[whh_t[:, (l * 3 + g) * H:(l * 3 + g + 1) * H] for g in range(3)]
                    for l in range(L)]

            def h_prev_ap(l, t, s):
                if t == 0:
                    return zro
                if l == L - 1:
                    return out2[:, (t - 1) * BLOC:t * BLOC]
                hc = [hA, hB][s % 2]
                return hc[:, l * BLOC:(l + 1) * BLOC]

            def h_new_ap(l, t, s):
                if l == L - 1:
                    return out2[:, t * BLOC:(t + 1) * BLOC]
                hn = [hA, hB][(s + 1) % 2]
                return hn[:, l * BLOC:(l + 1) * BLOC]

            # stage-major emission: within a superstep, issue each pipeline
            # stage for ALL active layers before the next stage, so every
            # engine's in-order queue interleaves the three independent
            # layer chains (no head-of-line blocking on a stalled chain).
            for s in range(SS):
                act = [l for l in range(L) if 0 <= s - l * D < T]
                ts_ = {l: s - l * D for l in act}
                tis = {l: ts_[l] % CH for l in act}
                hps = {l: h_prev_ap(l, ts_[l], s) for l in act}
                hns = {l: h_new_ap(l, ts_[l], s) for l in act}

                def emit_chunk_gemm(l, t0):
                    # gx GEMMs for steps [t0, t0+CH), drained to SBUF with the
                    # r/z biases folded into the ACT drain
                    cols = inps[l][:, t0 * BLOC:(t0 + CH) * BLOC]
                    gx = rsb.tile([128, 2, CH * BLOC], f32, bufs=2,
                                  tag=f"gxs{l}", name=f"gxs{l}_t{t0}")
                    for g in range(2):
                        nc.tensor.matmul(gcp[l][:, g, :], wihs[l][g], cols,
                                         start=True, stop=True)
                        nc.scalar.activation(
                            out=gx[:, g, :], in_=gcp[l][:, g, :],
                            func=AF.Identity,
                            bias=brz_t[:, 2 * l + g:2 * l + g + 1])
                    nc.tensor.matmul(xnsl[l], wihs[l][2], cols,
                                     start=True, stop=True)
                    return gx

                for l in act:          # first chunk of a layer (cold start)
                    if ts_[l] == 0:
                        gxs[l] = emit_chunk_gemm(l, 0)

                for l in act:          # a_rz: identity-inject gx, add Whh*h
                    nc.tensor.matmul(sxp[l][:, 0:2 * BLOC], ident2,
                                     gxs[l][:, :, tis[l] * BLOC:(tis[l] + 1) * BLOC],
                                     start=True, stop=False, skip_group_check=True)
                    for g in range(2):
                        nc.tensor.matmul(sxp[l][:, g * BLOC:(g + 1) * BLOC],
                                         whhs[l][g], hps[l],
                                         start=False, stop=(g == 1),
                                         skip_group_check=True)
                for l in act:          # hn matmuls (needed one stage later)
                    nc.tensor.matmul(hnsl[l], whhs[l][2], hps[l],
                                     start=True, stop=True)
                rz_s = {}
                for l in act:          # gates straight from PSUM
                    rz_s[l] = rsb.tile([128, 2 * BLOC], f32, tag=f"rzs{l}",
                                       name=f"rzs{l}_s{s}")
                    nc.scalar.activation(out=rz_s[l], in_=sxp[l][:, 0:2 * BLOC],
                                         func=AF.Sigmoid)
                t3 = {}
                for l in act:          # t3 = (hn + bhh_n) * r
                    t3[l] = rsb.tile([128, BLOC], f32, tag=f"t3{l}",
                                     name=f"t3{l}_s{s}")
                    nc.vector.scalar_tensor_tensor(
                        out=t3[l], in0=hnsl[l], scalar=bhn_t[:, l:l + 1],
                        in1=rz_s[l][:, 0:BLOC], op0=AL.add, op1=AL.mult)
                q2 = {}
                for l in act:          # q2 = z' * h  (off the tanh path, GPS)
                    q2[l] = rsb.tile([128, BLOC], f32, tag=f"q2{l}",
                                     name=f"q2{l}_s{s}")
                    nc.gpsimd.tensor_tensor(out=q2[l], in0=rz_s[l][:, BLOC:2 * BLOC],
                                            in1=hps[l], op=AL.mult)
                npre = {}
                for l in act:          # npre = (xn + bih_n) + t3
                    npre[l] = rsb.tile([128, BLOC], f32, tag=f"np{l}",
                                       name=f"np{l}_s{s}")
                    nc.vector.scalar_tensor_tensor(
                        out=npre[l],
                        in0=xnsl[l][:, tis[l] * BLOC:(tis[l] + 1) * BLOC],
                        scalar=bxn_t[:, l:l + 1],
                        in1=t3[l], op0=AL.add, op1=AL.add)
                pm = {}
                for l in act:          # p = h - q2  (off the tanh path, GPS)
                    pm[l] = rsb.tile([128, BLOC], f32, tag=f"pm{l}",
                                     name=f"pm{l}_s{s}")
                    nc.gpsimd.tensor_tensor(out=pm[l], in0=hps[l], in1=q2[l],
                                            op=AL.subtract)
                n_t = {}
                for l in act:          # n = tanh(npre)
                    n_t[l] = rsb.tile([128, BLOC], f32, tag=f"nt{l}",
                                      name=f"nt{l}_s{s}")
                    nc.scalar.activation(out=n_t[l], in_=npre[l], func=AF.Tanh)
                q1 = {}
                for l in act:          # q1 = z' * n
                    q1[l] = rsb.tile([128, BLOC], f32, tag=f"q1{l}",
                                     name=f"q1{l}_s{s}")
                    nc.vector.tensor_tensor(out=q1[l], in0=n_t[l],
                                            in1=rz_s[l][:, BLOC:2 * BLOC], op=AL.mult)
                for l in act:          # h' = p + q1 = (1-z')h + z'n
                    nc.vector.tensor_tensor(out=hns[l], in0=pm[l], in1=q1[l],
                                            op=AL.add)
                for l in act:          # residual into next layer's input
                    t = ts_[l]
                    if l < L - 1:
                        rl = rsb.tile([128, BLOC], f32, tag=f"rl{l}",
                                      name=f"rl{l}_s{s}")
                        nc.gpsimd.tensor_scalar_max(out=rl, in0=hns[l], scalar1=0.0)
                        nc.gpsimd.tensor_tensor(
                            out=inps[l + 1][:, t * BLOC:(t + 1) * BLOC],
                            in0=rl, in1=embT[:, t * BLOC:(t + 1) * BLOC], op=AL.add)

                for l in act:          # prefetch next chunk's gx (off-chain)
                    t = ts_[l]
                    if tis[l] == CH - 1 and t + 1 < T:
                        gxs[l] = emit_chunk_gemm(l, t + 1)

                if s % 16 == 4 and 1 <= (s + 12) // 16 <= 7:
                    emit_emb_piece((s + 12) // 16)

            # ---- final gather + tanh + transpose + store ----
            fg = rsb.tile([128, BLOC], f32, tag="fg")
            nc.gpsimd.ap_gather(fg, out2, fidx_t,
                                channels=128, num_elems=T * BLOC, d=1, num_idxs=BLOC)
            fth = rsb.tile([128, BLOC], f32, tag="fth")
            nc.scalar.activation(out=fth, in_=fg, func=AF.Tanh)
            pout = cps.tile([BLOC, H], f32, tag="hntp")
            nc.tensor.transpose(pout, fth, ident2)
            oux = rsb.tile([BLOC, H], f32, tag="oux")
            nc.vector.tensor_copy(oux, pout)
            nc.sync.dma_start(out=out_d[:, :], in_=oux)

    nc.finalize()
    return nc


def _host_prep(inputs):
    f32 = np.float32
    Hb = np.asarray(inputs["Hbat"], f32)
    DV2 = np.asarray(inputs["DV2"], f32)
    invDE = np.asarray(inputs["invDE"], f32)
    poi = np.asarray(inputs["poi"], f32)
    W1 = np.asarray(inputs["hgc1_w"], f32)
    b1 = np.asarray(inputs["hgc1_b"], f32)
    W2 = np.asarray(inputs["hgc2_w"], f32)
    b2 = np.asarray(inputs["hgc2_b"], f32)
    Wih = np.asarray(inputs["Wih"], f32)
    Whh = np.asarray(inputs["Whh"], f32)
    bih = np.asarray(inputs["bih"], f32)
    bhh = np.asarray(inputs["bhh"], f32)
    data = np.asarray(inputs["data"], np.int64)
    dlen = np.asarray(inputs["data_length"], np.int64)

    A_right = invDE[:, None] * Hb * DV2[None, :]              # [E, N]
    A_left_T = np.ascontiguousarray((DV2[:, None] * Hb.T).T)  # [E, N]
    rs8 = (A_right.sum(axis=1) / NC).astype(f32)              # [E]

    WIH = np.empty((H, L * 3 * H), f32)
    WHH = np.empty((H, L * 3 * H), f32)
    BRZ2 = np.empty((H, 2 * L), f32)
    BXN = np.empty((H, L), f32)
    BHN2 = np.empty((H, L), f32)
    for l in range(L):
        for g in range(3):
            sgn = -1.0 if g == 1 else 1.0
            WIH[:, (l * 3 + g) * H:(l * 3 + g + 1) * H] = \
                sgn * Wih[l][g * H:(g + 1) * H, :].T
            WHH[:, (l * 3 + g) * H:(l * 3 + g + 1) * H] = \
                sgn * Whh[l][g * H:(g + 1) * H, :].T
        BRZ2[:, 2 * l] = bih[l][:H] + bhh[l][:H]
        BRZ2[:, 2 * l + 1] = -(bih[l][H:2 * H] + bhh[l][H:2 * H])
        BXN[:, l] = bih[l][2 * H:]
        BHN2[:, l] = bhh[l][2 * H:]

    in_maps = []
    for c in range(NC):
        sl = slice(c * NLOC, (c + 1) * NLOC)
        seqs = slice(c * BLOC, (c + 1) * BLOC)
        dc = data[seqs]                                       # [64, T]
        eidx = np.empty(T * BLOC, np.int64)
        for t in range(T):
            eidx[t * BLOC:(t + 1) * BLOC] = dc[:, t]
        fidx = (dlen[seqs] - 1) * BLOC + np.arange(BLOC)
        in_maps.append(dict(
            R=np.ascontiguousarray(A_right.T[sl]),            # [NLOC, E]
            Lm=np.ascontiguousarray(A_left_T[:, sl]),         # [E, NLOC]
            poiT=np.ascontiguousarray(poi[sl].T),             # [H, NLOC]
            W1=W1, W2=W2,
            rsb=rs8.reshape(1, E),
            b1=b1.reshape(1, H), b2=b2.reshape(1, H),
            WIH=WIH, WHH=WHH, BRZ2=BRZ2, BXN=BXN, BHN2=BHN2,
            EIDX=_wrap_idx16(eidx), FIDX=_wrap_idx16(fidx),
        ))
    return in_maps


def _get_compiled():
    if "nc" not in _cache:
        _cache["nc"] = build()
    return _cache["nc"]


def run_on_cores(in_maps, **kw):
    from concourse.bass_utils import run_bass_kernel_spmd
    nc = _get_compiled()
    return run_bass_kernel_spmd(nc, in_maps, list(range(NC)), **kw)


def kernel(**inputs):
    in_maps = _host_prep(inputs)
    res = run_on_cores(in_maps)
    out = np.concatenate([res.results[c]["out"] for c in range(NC)], axis=0)
    return out.astype(np.float32)


# revision 29
# speedup vs baseline: 62.0487x; 62.0487x over previous
"""nn_HHLTraj Bass/Tile kernel for 8 trn2 NeuronCores (single NEFF).

Graph stage (hypergraph conv, factorized — G is never materialized):
    G @ Y = A_left @ (A_right @ Y),  A_left = DV2*Hbat^T [N,E],
    A_right = invDE*Hbat*DV2 [E,N].
  n-contraction sharded: core c holds A_right^T row-slice R_c [NLOC,E] and
  A_left^T col-slice L_c [E,NLOC]; z-partials are AllReduced ([H,E] 1MB),
  x-rows are computed locally, final x2^T is AllGathered. hgc biases enter
  as rank-1 matmuls (b ⊗ rowsum(A_right)/8), so Y1/Y2 GEMMs are bias-free.

GRU stage (3 layers, T=128 steps, B-sharded 64 seqs/core):
  layer wavefront with skew D supersteps; per-layer independent dependency
  chains (no cross-layer op merging). Gate preactivations accumulate in PSUM
  on top of the chunked gx GEMM output (no DVE add for r/z). z-gate is
  host-negated so z' = 1-z = sigmoid(-a_z) and h' = h + z'*(n - h).
  Per-step biases enter via ACT per-partition bias (sigmoid) and
  scalar_tensor_tensor per-partition scalars (n-gate). Elementwise is split
  across DVE / ACT / GPSIMD; gathers (emb, final step) use GPSIMD ap_gather
  against SBUF-resident x2^T / out2.

Numerics: all matmuls fp32 (bf16/tf32 operand rounding is amplified ~1000x
by this network — measured rel_err 0.5-2.0 vs the 2e-2 gate). fp32 h state
and out2 storage.
"""
import sys

import numpy as np

if "/opt/trn_rl_repo" not in sys.path:
    sys.path.insert(0, "/opt/trn_rl_repo")

N, E, B, T, H, L = 8192, 2048, 512, 128, 128, 3
NC = 8
NLOC = N // NC    # 1024
BLOC = B // NC    # 64
D = 5             # wavefront layer skew (supersteps)
CH = 4            # gx chunk length (steps)
SS = T + (L - 1) * D

_cache = {}


def _wrap_idx16(idx):
    """ap_gather index layout: idxs[p, s] = idx[s*16 + p%16] (all 8 Q7 groups
    get the same indices)."""
    idx = np.asarray(idx, np.int64)
    n = idx.shape[0]
    assert n % 16 == 0
    out = np.zeros((128, n // 16), np.int16)
    s = np.arange(n // 16)
    for p in range(128):
        out[p, :] = idx[s * 16 + (p % 16)]
    return out


def build(stub_collectives=False):
    import concourse.bacc as bacc
    import concourse.mybir as mybir
    import concourse.tile as tile
    from concourse.masks import make_identity

    f32 = mybir.dt.float32
    i16 = mybir.dt.int16

    nc = bacc.Bacc(num_devices=NC)

    # ---------------- DRAM parameters ----------------
    R_d = nc.declare_dram_parameter("R", [NLOC, E], f32, isOutput=False)
    L_d = nc.declare_dram_parameter("Lm", [E, NLOC], f32, isOutput=False)
    poiT_d = nc.declare_dram_parameter("poiT", [H, NLOC], f32, isOutput=False)
    W1_d = nc.declare_dram_parameter("W1", [H, H], f32, isOutput=False)
    W2_d = nc.declare_dram_parameter("W2", [H, H], f32, isOutput=False)
    rsb_d = nc.declare_dram_parameter("rsb", [1, E], f32, isOutput=False)
    b1_d = nc.declare_dram_parameter("b1", [1, H], f32, isOutput=False)
    b2_d = nc.declare_dram_parameter("b2", [1, H], f32, isOutput=False)
    wih_d = nc.declare_dram_parameter("WIH", [H, L * 3 * H], f32, isOutput=False)
    whh_d = nc.declare_dram_parameter("WHH", [H, L * 3 * H], f32, isOutput=False)
    brz_d = nc.declare_dram_parameter("BRZ2", [H, 2 * L], f32, isOutput=False)
    bxn_d = nc.declare_dram_parameter("BXN", [H, L], f32, isOutput=False)
    bhn_d = nc.declare_dram_parameter("BHN2", [H, L], f32, isOutput=False)
    eidx_d = nc.declare_dram_parameter("EIDX", [128, T * BLOC // 16], i16, isOutput=False)
    fidx_d = nc.declare_dram_parameter("FIDX", [128, BLOC // 16], i16, isOutput=False)
    out_d = nc.declare_dram_parameter("out", [BLOC, H], f32, isOutput=True)

    # internal DRAM for collectives
    zh1 = [(nc.dram_tensor(f"z1i{h}", [H, E // 2], f32),
            nc.dram_tensor(f"z1o{h}", [H, E // 2], f32, addr_space="Shared"))
           for h in range(2)]
    zh2 = [(nc.dram_tensor(f"z2i{h}", [H, E // 2], f32),
            nc.dram_tensor(f"z2o{h}", [H, E // 2], f32, addr_space="Shared"))
           for h in range(2)]
    ag_in = nc.dram_tensor("ag_in", [NLOC, H], f32)
    ag_out = nc.dram_tensor("ag_out", [N, H], f32, addr_space="Shared")

    rg = [list(range(NC))]
    AL = mybir.AluOpType
    AF = mybir.ActivationFunctionType

    with tile.TileContext(nc) as tc:
        # ================= graph stage =================
        with tc.tile_pool(name="gwt", bufs=1) as gwt, \
             tc.tile_pool(name="gbig", bufs=1) as gbig, \
             tc.tile_pool(name="gsb", bufs=1) as gsb, \
             tc.tile_pool(name="gps", bufs=2, space="PSUM") as gps, \
             tc.tile_pool(name="gpz", bufs=1, space="PSUM") as gpz:

            poiT_t = gwt.tile([H, NLOC], f32, tag="poiT")
            W1_t = gwt.tile([H, H], f32, tag="W1")
            W2_t = gwt.tile([H, H], f32, tag="W2")
            rsb_t = gwt.tile([1, E], f32, tag="rsb")
            b1_t = gwt.tile([1, H], f32, tag="b1")
            b2_t = gwt.tile([1, H], f32, tag="b2")
            ident = gwt.tile([H, H], f32, tag="ident")
            nc.sync.dma_start(out=poiT_t, in_=poiT_d[:, :])
            nc.sync.dma_start(out=W1_t, in_=W1_d[:, :])
            nc.sync.dma_start(out=W2_t, in_=W2_d[:, :])
            nc.sync.dma_start(out=rsb_t, in_=rsb_d[:, :])
            nc.sync.dma_start(out=b1_t, in_=b1_d[:, :])
            nc.sync.dma_start(out=b2_t, in_=b2_d[:, :])
            make_identity(nc, ident)

            Rt = gbig.tile([128, 8, E], f32, tag="Rt")
            Lt = gbig.tile([128, 16, NLOC], f32, tag="Lt")
            R_view = R_d.ap().rearrange("(k p) e -> p k e", p=128)
            L_view = L_d.ap().rearrange("(k p) n -> p k n", p=128)
            for k in range(8):
                nc.sync.dma_start(out=Rt[:, k, :], in_=R_view[:, k, :])
            for k in range(16):
                nc.sync.dma_start(out=Lt[:, k, :], in_=L_view[:, k, :])

            # Y1 = poi @ W1 (rows of this core, [128, 8, H], partition = n%128)
            Y1_t = gbig.tile([128, 8, H], f32, tag="Y1")
            for k in range(8):
                py = gps.tile([128, H], f32, tag="pp")
                nc.tensor.matmul(py, poiT_t[:, k * 128:(k + 1) * 128], W1_t,
                                 start=True, stop=True)
                nc.vector.tensor_copy(Y1_t[:, k, :], py)

            def g_multiply(Ysrc, btile, zhalves):
                # G @ Y with the z AllReduce split in E-halves so each AR
                # overlaps the other half's GEMM / transpose / x-GEMM work.
                pz = gpz.tile([128, E], f32, tag="pz")
                zT_sb = gsb.tile([H, E], f32, tag="zT")
                for h2 in range(2):
                    for j in range(2):
                        jj = h2 * 2 + j
                        for k in range(8):
                            nc.tensor.matmul(
                                pz[:, jj * 512:(jj + 1) * 512],
                                Ysrc[:, k, :], Rt[:, k, jj * 512:(jj + 1) * 512],
                                start=(k == 0), stop=False, skip_group_check=True)
                        nc.tensor.matmul(pz[:, jj * 512:(jj + 1) * 512], btile,
                                         rsb_t[:, jj * 512:(jj + 1) * 512],
                                         start=False, stop=True,
                                         skip_group_check=True)
                    nc.vector.tensor_copy(zT_sb[:, h2 * 1024:(h2 + 1) * 1024],
                                          pz[:, h2 * 1024:(h2 + 1) * 1024])
                    zin_dram, zout_dram = zhalves[h2]
                    nc.sync.dma_start(out=zin_dram[:, :],
                                      in_=zT_sb[:, h2 * 1024:(h2 + 1) * 1024])
                    if stub_collectives:
                        nc.sync.dma_start(out=zout_dram[:, :], in_=zin_dram[:, :])
                    else:
                        nc.gpsimd.collective_compute(
                            "AllReduce", AL.add, replica_groups=rg,
                            ins=[zin_dram.ap()], outs=[zout_dram.ap()])
                px = gpz.tile([128, NLOC], f32, tag="px")
                zt = gsb.tile([128, 16, H], f32, tag="zt")
                for h2 in range(2):
                    zTf = gsb.tile([H, E // 2], f32, tag="zTf", bufs=2,
                                   name=f"zTf_{h2}")
                    nc.sync.dma_start(out=zTf, in_=zhalves[h2][1][:, :])
                    for j in range(8):
                        pt = gps.tile([128, H], f32, tag="pp")
                        nc.tensor.transpose(pt, zTf[:, j * 128:(j + 1) * 128],
                                            ident)
                        k = h2 * 8 + j
                        if j % 2 == 0:
                            nc.vector.tensor_copy(zt[:, k, :], pt)
                        else:
                            nc.scalar.copy(zt[:, k, :], pt)
                    for k in range(h2 * 8, h2 * 8 + 8):
                        for j in range(2):
                            nc.tensor.matmul(
                                px[:, j * 512:(j + 1) * 512],
                                zt[:, k, :], Lt[:, k, j * 512:(j + 1) * 512],
                                start=(k == 0), stop=(k == 15),
                                skip_group_check=True)
                return px

            px1 = g_multiply(Y1_t, b1_t, zh1)
            # x1^T = relu(px1) + poi^T
            x1T = gsb.tile([H, NLOC], f32, tag="x1T")
            nc.vector.scalar_tensor_tensor(out=x1T, in0=px1, scalar=0.0,
                                           in1=poiT_t, op0=AL.max, op1=AL.add)
            # Y2 = x1 @ W2  [128, 8, H]
            Y2_t = gbig.tile([128, 8, H], f32, tag="Y2")
            for k in range(8):
                py = gps.tile([128, H], f32, tag="pp")
                nc.tensor.matmul(py, x1T[:, k * 128:(k + 1) * 128], W2_t,
                                 start=True, stop=True)
                nc.vector.tensor_copy(Y2_t[:, k, :], py)
            px2 = g_multiply(Y2_t, b2_t, zh2)
            x2T_c = gsb.tile([H, NLOC], f32, tag="x1T")
            for j in range(2):
                nc.vector.tensor_copy(x2T_c[:, j * 512:(j + 1) * 512],
                                      px2[:, j * 512:(j + 1) * 512])
            # transpose to row-major x2 [NLOC, H] so emb rows can be
            # dma-gathered from DRAM after the AllGather
            x2row = gsb.tile([128, 8, H], f32, tag="zt")
            for j in range(8):
                pt = gps.tile([128, H], f32, tag="pp")
                nc.tensor.transpose(pt, x2T_c[:, j * 128:(j + 1) * 128], ident)
                if j % 2 == 0:
                    nc.vector.tensor_copy(x2row[:, j, :], pt)
                else:
                    nc.scalar.copy(x2row[:, j, :], pt)
            nc.sync.dma_start(
                out=ag_in.ap().rearrange("(j p) h -> p j h", p=128), in_=x2row)
            if stub_collectives:
                for c in range(NC):
                    nc.sync.dma_start(out=ag_out[c * NLOC:(c + 1) * NLOC, :],
                                      in_=ag_in[:, :])
            else:
                nc.gpsimd.collective_compute(
                    "AllGather", AL.bypass, replica_groups=rg,
                    ins=[ag_in.ap()], outs=[ag_out.ap()])

        # ================= GRU stage =================
        with tc.tile_pool(name="rwt", bufs=1) as rwt, \
             tc.tile_pool(name="rbig", bufs=1) as rbig, \
             tc.tile_pool(name="rsb", bufs=4) as rsb, \
             tc.tile_pool(name="cps", bufs=1, space="PSUM") as cps:

            wih_t = rwt.tile([H, L * 3 * H], f32, tag="wih")
            whh_t = rwt.tile([H, L * 3 * H], f32, tag="whh")
            brz_t = rwt.tile([H, 2 * L], f32, tag="brz")
            bxn_t = rwt.tile([H, L], f32, tag="bxn")
            bhn_t = rwt.tile([H, L], f32, tag="bhn")
            eidx_t = rwt.tile([128, T * BLOC // 16], i16, tag="eidx")
            fidx_t = rwt.tile([128, BLOC // 16], i16, tag="fidx")
            nc.sync.dma_start(out=wih_t, in_=wih_d[:, :])
            nc.sync.dma_start(out=whh_t, in_=whh_d[:, :])
            nc.sync.dma_start(out=brz_t, in_=brz_d[:, :])
            nc.sync.dma_start(out=bxn_t, in_=bxn_d[:, :])
            nc.sync.dma_start(out=bhn_t, in_=bhn_d[:, :])
            nc.sync.dma_start(out=eidx_t, in_=eidx_d[:, :])
            nc.sync.dma_start(out=fidx_t, in_=fidx_d[:, :])

            embT = rbig.tile([128, T * BLOC], f32, tag="embT")
            inp1 = rbig.tile([128, T * BLOC], f32, tag="inp1")
            inp2 = rbig.tile([128, T * BLOC], f32, tag="inp2")
            out2 = rbig.tile([128, T * BLOC], f32, tag="out2")
            inps = [embT, inp1, inp2]

            ident2 = rwt.tile([H, H], f32, tag="ident2")
            make_identity(nc, ident2)

            hA = rwt.tile([128, 2 * BLOC], f32, tag="hA")   # layers 0,1 ping
            hB = rwt.tile([128, 2 * BLOC], f32, tag="hB")   # layers 0,1 pong
            zro = rwt.tile([128, BLOC], f32, tag="zro")
            nc.vector.memset(hA, 0.0)
            nc.vector.memset(hB, 0.0)
            nc.vector.memset(zro, 0.0)

            # persistent PSUM (7 banks):
            #  - per-layer gx-rz chunk gcp [1 bank each]
            #  - per-layer step tile sxp = [a_rz(128) | xn-chunk(256)] [1 bank]
            #  - hntp = [hn0|hn1|hn2(192) | transpose scratch(128)] [1 bank]
            # gx is injected into a_rz by an identity matmul, so the sigmoid
            # reads the complete preactivation straight from PSUM (no DVE add)
            gcp = [cps.tile([128, 2, CH * BLOC], f32, tag=f"gc{l}", name=f"gcp{l}")
                   for l in range(L)]
            sxp = [cps.tile([128, 2 * BLOC + CH * BLOC], f32, tag=f"sx{l}",
                            name=f"sxp{l}") for l in range(L)]
            hntp = cps.tile([128, L * BLOC + H], f32, tag="hntp")
            tpp = hntp[:, L * BLOC:L * BLOC + H]
            hnsl = [hntp[:, l * BLOC:(l + 1) * BLOC] for l in range(L)]
            xnsl = [sxp[l][:, 2 * BLOC:] for l in range(L)]

            def emit_emb_piece(g):
                # gather 1024 emb rows from row-major x2 in DRAM, then
                # transpose 128-row blocks into embT columns
                er = rbig.tile([128, 8, H], f32, bufs=2, tag="embr",
                               name=f"embr{g}")
                nc.gpsimd.dma_gather(er, ag_out.ap(),
                                     eidx_t[:, g * 64:(g + 1) * 64],
                                     num_idxs=1024, num_idxs_reg=1024,
                                     elem_size=H)
                for j in range(8):
                    nc.tensor.transpose(tpp, er[:, j, :], ident2)
                    dst = embT[:, g * 1024 + j * 128:g * 1024 + (j + 1) * 128]
                    if j % 2 == 0:
                        nc.vector.tensor_copy(dst, tpp)
                    else:
                        nc.scalar.copy(dst, tpp)

            emit_emb_piece(0)

            gxs = [None] * L
            wihs = [[wih_t[:, (l * 3 + g) * H:(l * 3 + g + 1) * H] for g in range(3)]
                    for l in range(L)]
            whhs = [[whh_t[:, (l * 3 + g) * H:(l * 3 + g + 1) * H] for g in range(3)]
                    for l in range(L)]

            def h_prev_ap(l, t, s):
                if t == 0:
                    return zro
                if l == L - 1:
                    return out2[:, (t - 1) * BLOC:t * BLOC]
                hc = [hA, hB][s % 2]
                return hc[:, l * BLOC:(l + 1) * BLOC]

            def h_new_ap(l, t, s):
                if l == L - 1:
                    return out2[:, t * BLOC:(t + 1) * BLOC]
                hn = [hA, hB][(s + 1) % 2]
                return hn[:, l * BLOC:(l + 1) * BLOC]

            # stage-major emission: within a superstep, issue each pipeline
            # stage for ALL active layers before the next stage, so every
            # engine's in-order queue interleaves the three independent
            # layer chains (no head-of-line blocking on a stalled chain).
            for s in range(SS):
                act = [l for l in range(L) if 0 <= s - l * D < T]
                ts_ = {l: s - l * D for l in act}
                tis = {l: ts_[l] % CH for l in act}
                hps = {l: h_prev_ap(l, ts_[l], s) for l in act}
                hns = {l: h_new_ap(l, ts_[l], s) for l in act}

                def emit_chunk_gemm(l, t0):
                    # gx GEMMs for steps [t0, t0+CH), drained to SBUF with the
                    # r/z biases folded into the ACT drain
                    cols = inps[l][:, t0 * BLOC:(t0 + CH) * BLOC]
                    gx = rsb.tile([128, 2, CH * BLOC], f32, bufs=2,
                                  tag=f"gxs{l}", name=f"gxs{l}_t{t0}")
                    for g in range(2):
                        nc.tensor.matmul(gcp[l][:, g, :], wihs[l][g], cols,
                                         start=True, stop=True)
                        nc.scalar.activation(
                            out=gx[:, g, :], in_=gcp[l][:, g, :],
                            func=AF.Identity,
                            bias=brz_t[:, 2 * l + g:2 * l + g + 1])
                    nc.tensor.matmul(xnsl[l], wihs[l][2], cols,
                                     start=True, stop=True)
                    return gx

                for l in act:          # first chunk of a layer (cold start)
                    if ts_[l] == 0:
                        gxs[l] = emit_chunk_gemm(l, 0)

                for l in act:          # a_rz: identity-inject gx, add Whh*h
                    nc.tensor.matmul(sxp[l][:, 0:2 * BLOC], ident2,
                                     gxs[l][:, :, tis[l] * BLOC:(tis[l] + 1) * BLOC],
                                     start=True, stop=False, skip_group_check=True)
                    for g in range(2):
                        nc.tensor.matmul(sxp[l][:, g * BLOC:(g + 1) * BLOC],
                                         whhs[l][g], hps[l],
                                         start=False, stop=(g == 1),
                                         skip_group_check=True)
                for l in act:          # hn matmuls (needed one stage later)
                    nc.tensor.matmul(hnsl[l], whhs[l][2], hps[l],
                                     start=True, stop=True)
                rz_s = {}
                for l in act:          # gates straight from PSUM
                    rz_s[l] = rsb.tile([128, 2 * BLOC], f32, tag=f"rzs{l}",
                                       name=f"rzs{l}_s{s}")
                    nc.scalar.activation(out=rz_s[l], in_=sxp[l][:, 0:2 * BLOC],
                                         func=AF.Sigmoid)
                t3 = {}
                for l in act:          # t3 = (hn + bhh_n) * r
                    t3[l] = rsb.tile([128, BLOC], f32, tag=f"t3{l}",
                                     name=f"t3{l}_s{s}")
                    nc.vector.scalar_tensor_tensor(
                        out=t3[l], in0=hnsl[l], scalar=bhn_t[:, l:l + 1],
                        in1=rz_s[l][:, 0:BLOC], op0=AL.add, op1=AL.mult)
                q2 = {}
                for l in act:          # q2 = z' * h  (off the tanh path, GPS)
                    q2[l] = rsb.tile([128, BLOC], f32, tag=f"q2{l}",
                                     name=f"q2{l}_s{s}")
                    nc.gpsimd.tensor_tensor(out=q2[l], in0=rz_s[l][:, BLOC:2 * BLOC],
                                            in1=hps[l], op=AL.mult)
                npre = {}
                for l in act:          # npre = (xn + bih_n) + t3
                    npre[l] = rsb.tile([128, BLOC], f32, tag=f"np{l}",
                                       name=f"np{l}_s{s}")
                    nc.vector.scalar_tensor_tensor(
                        out=npre[l],
                        in0=xnsl[l][:, tis[l] * BLOC:(tis[l] + 1) * BLOC],
                        scalar=bxn_t[:, l:l + 1],
                        in1=t3[l], op0=AL.add, op1=AL.add)
                pm = {}
                for l in act:          # p = h - q2  (off the tanh path, GPS)
                    pm[l] = rsb.tile([128, BLOC], f32, tag=f"pm{l}",
                                     name=f"pm{l}_s{s}")
                    nc.gpsimd.tensor_tensor(out=pm[l], in0=hps[l], in1=q2[l],
                                            op=AL.subtract)
                n_t = {}
                for l in act:          # n = tanh(npre)
                    n_t[l] = rsb.tile([128, BLOC], f32, tag=f"nt{l}",
                                      name=f"nt{l}_s{s}")
                    nc.scalar.activation(out=n_t[l], in_=npre[l], func=AF.Tanh)
                q1 = {}
                for l in act:          # q1 = z' * n
                    q1[l] = rsb.tile([128, BLOC], f32, tag=f"q1{l}",
                                     name=f"q1{l}_s{s}")
                    nc.vector.tensor_tensor(out=q1[l], in0=n_t[l],
                                            in1=rz_s[l][:, BLOC:2 * BLOC], op=AL.mult)
                for l in act:          # h' = p + q1 = (1-z')h + z'n
                    nc.vector.tensor_tensor(out=hns[l], in0=pm[l], in1=q1[l],
                                            op=AL.add)
                for l in act:          # residual into next layer's input
                    t = ts_[l]
                    if l < L - 1:
                        rl = rsb.tile([128, BLOC], f32, tag=f"rl{l}",
                                      name=f"rl{l}_s{s}")
                        nc.gpsimd.tensor_scalar_max(out=rl, in0=hns[l], scalar1=0.0)
                        nc.gpsimd.tensor_tensor(
                            out=inps[l + 1][:, t * BLOC:(t + 1) * BLOC],
                            in0=rl, in1=embT[:, t * BLOC:(t + 1) * BLOC], op=AL.add)

                for l in act:          # prefetch next chunk's gx (off-chain)
                    t = ts_[l]
                    if tis[l] == CH - 1 and t + 1 < T:
                        gxs[l] = emit_chunk_gemm(l, t + 1)

                if s % 16 == 4 and 1 <= (s + 12) // 16 <= 7:
                    emit_emb_piece((s + 12) // 16)

            # ---- final gather + tanh + transpose + store ----
            fg = rsb.tile([128, BLOC], f32, tag="fg")
            nc.gpsimd.ap_gather(fg, out2, fidx_t,
                                channels=128, num_elems=T * BLOC, d=1, num_idxs=BLOC)
            fth = rsb.tile([128, BLOC], f32, tag="fth")
            nc.scalar.activation(out=fth, in_=fg, func=AF.Tanh)
            pout = cps.tile([BLOC, H], f32, tag="hntp")
            nc.tensor.transpose(pout, fth, ident2)
            oux = rsb.tile([BLOC, H], f32, tag="oux")
            nc.vector.tensor_copy(oux, pout)
            nc.sync.dma_start(out=out_d[:, :], in_=oux)

    nc.finalize()
    return nc


def _host_prep(inputs):
    f32 = np.float32
    Hb = np.asarray(inputs["Hbat"], f32)
    DV2 = np.asarray(inputs["DV2"], f32)
    invDE = np.asarray(inputs["invDE"], f32)
    poi = np.asarray(inputs["poi"], f32)
    W1 = np.asarray(inputs["hgc1_w"], f32)
    b1 = np.asarray(inputs["hgc1_b"], f32)
    W2 = np.asarray(inputs["hgc2_w"], f32)
    b2 = np.asarray(inputs["hgc2_b"], f32)
    Wih = np.asarray(inputs["Wih"], f32)
    Whh = np.asarray(inputs["Whh"], f32)
    bih = np.asarray(inputs["bih"], f32)
    bhh = np.asarray(inputs["bhh"], f32)
    data = np.asarray(inputs["data"], np.int64)
    dlen = np.asarray(inputs["data_length"], np.int64)

    A_right = invDE[:, None] * Hb * DV2[None, :]              # [E, N]
    A_left_T = np.ascontiguousarray((DV2[:, None] * Hb.T).T)  # [E, N]
    rs8 = (A_right.sum(axis=1) / NC).astype(f32)              # [E]

    WIH = np.empty((H, L * 3 * H), f32)
    WHH = np.empty((H, L * 3 * H), f32)
    BRZ2 = np.empty((H, 2 * L), f32)
    BXN = np.empty((H, L), f32)
    BHN2 = np.empty((H, L), f32)
    for l in range(L):
        for g in range(3):
            sgn = -1.0 if g == 1 else 1.0
            WIH[:, (l * 3 + g) * H:(l * 3 + g + 1) * H] = \
                sgn * Wih[l][g * H:(g + 1) * H, :].T
            WHH[:, (l * 3 + g) * H:(l * 3 + g + 1) * H] = \
                sgn * Whh[l][g * H:(g + 1) * H, :].T
        BRZ2[:, 2 * l] = bih[l][:H] + bhh[l][:H]
        BRZ2[:, 2 * l + 1] = -(bih[l][H:2 * H] + bhh[l][H:2 * H])
        BXN[:, l] = bih[l][2 * H:]
        BHN2[:, l] = bhh[l][2 * H:]

    in_maps = []
    for c in range(NC):
        sl = slice(c * NLOC, (c + 1) * NLOC)
        seqs = slice(c * BLOC, (c + 1) * BLOC)
        dc = data[seqs]                                       # [64, T]
        eidx = np.empty(T * BLOC, np.int64)
        for t in range(T):
            eidx[t * BLOC:(t + 1) * BLOC] = dc[:, t]
        fidx = (dlen[seqs] - 1) * BLOC + np.arange(BLOC)
        in_maps.append(dict(
            R=np.ascontiguousarray(A_right.T[sl]),            # [NLOC, E]
            Lm=np.ascontiguousarray(A_left_T[:, sl]),         # [E, NLOC]
            poiT=np.ascontiguousarray(poi[sl].T),             # [H, NLOC]
            W1=W1, W2=W2,
            rsb=rs8.reshape(1, E),
            b1=b1.reshape(1, H), b2=b2.reshape(1, H),
            WIH=WIH, WHH=WHH, BRZ2=BRZ2, BXN=BXN, BHN2=BHN2,
            EIDX=_wrap_idx16(eidx), FIDX=_wrap_idx16(fidx),
        ))
    return in_maps


def _get_compiled():
    if "nc" not in _cache:
        _cache["nc"] = build()
    return _cache["nc"]


def run_on_cores(in_maps, **kw):
    from concourse.bass_utils import run_bass_kernel_spmd
    nc = _get_compiled()
    return run_bass_kernel_spmd(nc, in_maps, list(range(NC)), **kw)


def kernel(**inputs):
    in_maps = _host_prep(inputs)
    res = run_on_cores(in_maps)
    out = np.concatenate([res.results[c]["out"] for c in range(NC)], axis=0)
    return out.astype(np.float32)
